# revision 16
# baseline (speedup 1.0000x reference)
"""AFNO block on 8 trn2 NeuronCores.

Sharding: core c -> (batch b = c//2, h-half = c%2). Each core runs the
spectral path (LN1 + rfft2 + block-MLP + irfft2) for its whole batch image
(redundant with its pair core) and the token-path (residual + LN2 + MLP)
for its 32 h-rows only. The h-row selection is baked into per-core iFFT-h
matrices passed as inputs, so all cores execute one identical SPMD graph.
No collectives.

FFTs are exact DFT matmuls (matrices derived from numpy fft on basis
vectors). Matmuls run in bf16 with fp32 PSUM accumulation.
"""
import os
import sys

for _p in ("/opt/trn_rl_repo", "/root/.axon_site/_ro/trn_rl_repo"):
    if os.path.isdir(_p) and _p not in sys.path:
        sys.path.insert(0, _p)

import numpy as np
import concourse.bass as bass
import concourse.tile as tile
from concourse import mybir
from concourse.bass_utils import run_bass_kernel_spmd
from concourse.masks import make_identity
from concourse.vector_clock import ScopedClock

H, W, DIM, NB, BS = 64, 128, 768, 8, 96
Wf = W // 2 + 1          # 65
LAM, EPS = 0.01, 1e-5
HH = H // 2              # 32 h-rows per core
NTOK = HH * W            # 4096 tokens per core
NPOS = H * Wf            # 4160 spectral positions
FLATZ = Wf * DIM         # 49920
BF = mybir.dt.bfloat16
F32 = mybir.dt.float32

# ---------------------------------------------------------------- tile patch
_MAX_SP_WAITS = 1


def _patched_drain_and_barrier(self, tick_clock, wait_clock):
    nc = self.nc
    sc = ScopedClock({None: tick_clock.global_clock})
    probe = nc.sync.nop(nofuse=True)
    wait_clock.add_sem_waits(probe.ins, sc)
    si = probe.ins.sync_info
    waits = list(si.on_wait) if si is not None else []
    si.on_wait = waits[:_MAX_SP_WAITS]
    probe.ins.sync_info = si
    rest = waits[_MAX_SP_WAITS:]
    while rest:
        chunk, rest = rest[:_MAX_SP_WAITS], rest[_MAX_SP_WAITS:]
        n = nc.sync.nop(nofuse=True)
        wait_clock.add_sem_waits(n.ins, sc)
        nsi = n.ins.sync_info
        nsi.on_wait = chunk
        n.ins.sync_info = nsi
    nc.sync.drain()
    nc.all_engine_barrier()
    popped = nc._tile_sem_poison_stack.pop()
    assert popped is self._sem_poison
    nc.clear_and_free_semaphores(list(self.sems.allocated().values()))
    nc.all_engine_barrier()


tile.TileContext._drain_and_barrier = _patched_drain_and_barrier


def _nchunks(total, step):
    out = []
    o = 0
    while o < total:
        out.append((o, min(step, total - o)))
        o += step
    return out


def build_graph():
    nc = bass.Bass("TRN2", target_bir_lowering=False)
    P = lambda name, shape: nc.declare_dram_parameter(name, list(shape), F32, isOutput=False)

    x_ext = P("x", [H * W, DIM])          # full batch image
    xres_ext = P("xres", [NTOK, DIM])     # this core's token rows
    fwr_ext = P("fwr", [W, Wf])
    fwi_ext = P("fwi", [W, Wf])
    ga_ext = P("ga", [H, 2 * H])          # [Gr | Gi]
    gb_ext = P("gb", [H, 2 * H])          # [-Gi | Gr]
    gc_ext = P("gc", [H, 2 * HH])         # per-core [Gr_sel | -Gi_sel]
    gd_ext = P("gd", [H, 2 * HH])         # per-core [Gi_sel | Gr_sel]
    br_ext = P("brm", [Wf, W])
    bi_ext = P("bim", [Wf, W])
    w1_ext = P("w1", [NB, BS, BS])
    w2_ext = P("w2", [NB, BS, BS])
    b1t_ext = P("b1t", [BS, NB])
    b2mt_ext = P("b2mt", [BS, NB])        # b2.T - lam
    b2pt_ext = P("b2pt", [BS, NB])        # -b2.T - lam
    ln1w_ext = P("ln1w", [DIM])
    ln1b_ext = P("ln1b", [DIM])
    ln2w_ext = P("ln2w", [DIM])
    ln2b_ext = P("ln2b", [DIM])
    mb2_ext = P("mb2", [DIM])
    mb1_ext = P("mb1", [4 * DIM])
    mw1_ext = P("mw1", [DIM, 4 * DIM])
    mw2_ext = P("mw2", [4 * DIM, DIM])
    out_ext = nc.declare_dram_parameter("out", [NTOK, DIM], F32, isOutput=True)

    # DRAM intermediates
    Zd = nc.dram_tensor("Zd", [H, 2 * Wf, DIM], BF)       # w-fft out (r|i on axis1)
    Yd = nc.dram_tensor("Yd", [2 * H, FLATZ], BF)         # h-fft out (k1 r|i rows)
    Od = nc.dram_tensor("Od", [2, H, Wf, DIM], BF)        # block-mlp out
    Ad = nc.dram_tensor("Ad", [2 * HH, FLATZ], BF)        # ifft-h out (sel h rows r|i)

    with tile.TileContext(nc) as tc:
        with (
            tc.tile_pool(name="consts", bufs=1) as CP,
            tc.tile_pool(name="ctmp", bufs=1) as CT,
            tc.tile_pool(name="work", bufs=2) as WK,
            tc.tile_pool(name="zout", bufs=3) as ZO,
            tc.tile_pool(name="srhs", bufs=3) as SR,
            tc.tile_pool(name="s3", bufs=6) as S3,
            tc.tile_pool(name="h1p", bufs=24) as H1P,
            tc.tile_pool(name="xsp", bufs=5) as XSP,
            tc.tile_pool(name="xtg", bufs=1) as XTG,
            tc.tile_pool(name="outp", bufs=4) as OP,
            tc.tile_pool(name="ps_big", bufs=2, space="PSUM") as PB,
            tc.tile_pool(name="ps_mm", bufs=3, space="PSUM") as PM,
        ):
            # ---------------- constants -> SBUF (bf16 where matmul operand)
            def load_bf(ext, shape, tag):
                t32 = CT.tile(list(shape), F32, tag="ctmp32")
                nc.gpsimd.dma_start(out=t32, in_=ext[:])
                tb = CP.tile(list(shape), BF, tag=tag)
                nc.vector.tensor_copy(out=tb, in_=t32)
                return tb

            fwr_sb = load_bf(fwr_ext, [W, Wf], "c_fwr")
            fwi_sb = load_bf(fwi_ext, [W, Wf], "c_fwi")
            ga_sb = load_bf(ga_ext, [H, 2 * H], "c_ga")
            gb_sb = load_bf(gb_ext, [H, 2 * H], "c_gb")
            gc_sb = load_bf(gc_ext, [H, 2 * HH], "c_gc")
            gd_sb = load_bf(gd_ext, [H, 2 * HH], "c_gd")
            br_sb = load_bf(br_ext, [Wf, W], "c_br")
            bi_sb = load_bf(bi_ext, [Wf, W], "c_bi")

            # block weights as [j, n, m]
            w1_sb = CP.tile([BS, NB, BS], BF)
            w2_sb = CP.tile([BS, NB, BS], BF)
            for ext, sb in ((w1_ext, w1_sb), (w2_ext, w2_sb)):
                t32 = CT.tile([BS, NB, BS], F32, tag="ctmp32")
                nc.gpsimd.dma_start(out=t32, in_=ext.ap().rearrange("n j m -> j n m"))
                nc.vector.tensor_copy(out=sb, in_=t32)

            b1t_sb = CP.tile([BS, NB], F32)
            nc.gpsimd.dma_start(out=b1t_sb, in_=b1t_ext[:])
            b2mt_sb = CP.tile([BS, NB], F32)
            nc.gpsimd.dma_start(out=b2mt_sb, in_=b2mt_ext[:])
            b2pt_sb = CP.tile([BS, NB], F32)
            nc.gpsimd.dma_start(out=b2pt_sb, in_=b2pt_ext[:])

            def bcast(ext, n, tag):
                t32 = CT.tile([128, n], F32, tag="ctmp32b")
                a = ext.ap()
                ap = bass.AP(tensor=a.tensor, offset=a.offset,
                             ap=[[0, 128]] + list(a.ap))
                nc.gpsimd.dma_start(out=t32, in_=ap)
                t = CP.tile([128, n], BF, tag=tag)
                nc.vector.tensor_copy(out=t, in_=t32)
                return t

            ln1w_bc = bcast(ln1w_ext, DIM, "c_l1w")
            ln1b_bc = bcast(ln1b_ext, DIM, "c_l1b")
            ln2w_bc = bcast(ln2w_ext, DIM, "c_l2w")
            ln2b_bc = bcast(ln2b_ext, DIM, "c_l2b")
            mb2_bc = bcast(mb2_ext, DIM, "c_mb2")

            mb1_sb = CP.tile([128, 24], F32)
            _a = mb1_ext.ap()
            nc.gpsimd.dma_start(
                out=mb1_sb,
                in_=bass.AP(tensor=_a.tensor, offset=_a.offset,
                            ap=[[1, 128], [128, 24]]))

            eps_sb = CP.tile([128, 1], F32)
            nc.vector.memset(eps_sb, EPS)
            ident = CP.tile([128, 128], BF)
            make_identity(nc, ident)

            mw1_sb = []
            for cc in range(6):
                tb = CP.tile([128, 4 * DIM], BF, tag=f"c_mw1_{cc}")
                for hhalf in range(2):
                    t32 = CT.tile([128, 2 * DIM], F32, tag="mwtmp")
                    nc.gpsimd.dma_start(
                        out=t32,
                        in_=mw1_ext[cc * 128:(cc + 1) * 128,
                                    hhalf * 2 * DIM:(hhalf + 1) * 2 * DIM])
                    nc.vector.tensor_copy(
                        out=tb[:, hhalf * 2 * DIM:(hhalf + 1) * 2 * DIM], in_=t32)
                mw1_sb.append(tb)
            mw2_sb = []
            for mm_ in range(24):
                t32 = CT.tile([128, DIM], F32, tag="mwtmp2")
                nc.gpsimd.dma_start(out=t32, in_=mw2_ext[mm_ * 128:(mm_ + 1) * 128, :])
                tb = CP.tile([128, DIM], BF, tag=f"c_mw2_{mm_}")
                nc.vector.tensor_copy(out=tb, in_=t32)
                mw2_sb.append(tb)

            # ---------------- helpers
            def layernorm(xt, w_bc, b_bc, out_bf):
                """xt [128, DIM] f32 inplace-normalized, scaled, cast to out_bf."""
                xg = xt.rearrange("p (s f) -> p s f", f=256)
                st = WK.tile([128, 3, 6], F32, tag="bnst")
                for s in range(3):
                    nc.vector.bn_stats(out=st[:, s, :], in_=xg[:, s, :])
                mv = WK.tile([128, 2], F32, tag="bnmv")
                nc.vector.bn_aggr(out=mv, in_=st)
                rstd = WK.tile([128, 1], F32, tag="bnrs")
                nc.scalar.activation(out=rstd, in_=mv[:, 1:2],
                                     func=mybir.ActivationFunctionType.Sqrt,
                                     bias=eps_sb, scale=1.0)
                nc.vector.reciprocal(out=rstd, in_=rstd)
                nc.vector.tensor_scalar(out=xt, in0=xt, scalar1=mv[:, 0:1],
                                        scalar2=rstd,
                                        op0=mybir.AluOpType.subtract,
                                        op1=mybir.AluOpType.mult)
                nc.vector.tensor_mul(out=xt, in0=xt, in1=w_bc)
                nc.vector.tensor_add(out=xt, in0=xt, in1=b_bc)
                nc.scalar.copy(out=out_bf, in_=xt)

            # ================= S1: LN1 + w-FFT, full image =================
            Zdv = Zd.ap()
            for h in range(H):
                xt = WK.tile([128, DIM], F32, tag="xln")
                nc.sync.dma_start(out=xt, in_=x_ext[h * W:(h + 1) * W, :])
                xnb = WK.tile([128, DIM], BF, tag="xnb")
                layernorm(xt, ln1w_bc, ln1b_bc, xnb)
                zr_ps = PB.tile([Wf, DIM], F32, tag="big")
                zi_ps = PB.tile([Wf, DIM], F32, tag="big")
                for (o, n) in ((0, 512), (512, 256)):
                    nc.tensor.matmul(zr_ps[:, o:o + n], fwr_sb, xnb[:, o:o + n],
                                     start=True, stop=True)
                for (o, n) in ((0, 512), (512, 256)):
                    nc.tensor.matmul(zi_ps[:, o:o + n], fwi_sb, xnb[:, o:o + n],
                                     start=True, stop=True)
                zr_sb = ZO.tile([Wf, DIM], BF, tag="zsb")
                zi_sb = ZO.tile([Wf, DIM], BF, tag="zsb")
                nc.scalar.copy(out=zr_sb, in_=zr_ps)
                nc.scalar.copy(out=zi_sb, in_=zi_ps)
                nc.sync.dma_start(out=Zdv[h, 0:Wf, :], in_=zr_sb)
                nc.sync.dma_start(out=Zdv[h, Wf:2 * Wf, :], in_=zi_sb)

            # ================= S2: h-FFT ===================================
            Zflat = Zd.ap().rearrange("h r d -> h (r d)")   # [H, 2*FLATZ]
            for (o, n) in _nchunks(FLATZ, 512):
                rr = SR.tile([H, n], BF, tag="s2r")
                ri = SR.tile([H, n], BF, tag="s2r")
                nc.sync.dma_start(out=rr, in_=Zflat[:, o:o + n])
                nc.sync.dma_start(out=ri, in_=Zflat[:, FLATZ + o:FLATZ + o + n])
                ps = PM.tile([128, 512], F32, tag="mm")
                nc.tensor.matmul(ps[:, 0:n], ga_sb, rr, start=True, stop=False)
                nc.tensor.matmul(ps[:, 0:n], gb_sb, ri, start=False, stop=True)
                yb = ZO.tile([128, 512], BF, tag="ysb")
                nc.scalar.copy(out=yb[:, 0:n], in_=ps[:, 0:n])
                nc.sync.dma_start(out=Yd.ap()[:, o:o + n], in_=yb[:, 0:n])

            # ================= S3: block MLP ===============================
            Yv = Yd.ap().rearrange("a (l c) -> a l c", l=Wf)    # [2H, Wf, DIM]
            Ov = Od.ap()                                        # [2, H, Wf, DIM]
            half = NPOS // 2                                    # 2080
            for nb in range(NB):
                c0 = nb * BS
                for ph in range(2):                              # position halves
                    h0, h1_ = (0, H // 2) if ph == 0 else (H // 2, H)
                    yr = S3.tile([BS, H // 2, Wf], BF, tag="s3t")
                    yi = S3.tile([BS, H // 2, Wf], BF, tag="s3t")
                    nc.sync.dma_start(
                        out=yr, in_=Yv[h0:h1_, :, c0:c0 + BS].rearrange("k l j -> j k l"))
                    nc.sync.dma_start(
                        out=yi, in_=Yv[H + h0:H + h1_, :, c0:c0 + BS].rearrange("k l j -> j k l"))
                    u = S3.tile([BS, H // 2, Wf], BF, tag="s3t")
                    v = S3.tile([BS, H // 2, Wf], BF, tag="s3t")
                    nc.vector.tensor_sub(out=u, in0=yr, in1=yi)
                    nc.vector.tensor_add(out=v, in0=yr, in1=yi)
                    uf = u.rearrange("j k l -> j (k l)")
                    vf = v.rearrange("j k l -> j (k l)")
                    o1r = S3.tile([BS, half], BF, tag="s3t")
                    o1i = S3.tile([BS, half], BF, tag="s3t")
                    for (src, dst) in ((uf, o1r), (vf, o1i)):
                        for (o, n) in _nchunks(half, 512):
                            ps = PM.tile([128, 512], F32, tag="mm")
                            nc.tensor.matmul(ps[:BS, 0:n], w1_sb[:, nb, :],
                                             src[:, o:o + n], start=True, stop=True)
                            nc.scalar.activation(out=dst[:, o:o + n], in_=ps[:BS, 0:n],
                                                 func=mybir.ActivationFunctionType.Relu,
                                                 bias=b1t_sb[:, nb:nb + 1], scale=1.0)
                    p = S3.tile([BS, half], BF, tag="s3t")
                    q = S3.tile([BS, half], BF, tag="s3t")
                    nc.vector.tensor_sub(out=p, in0=o1r, in1=o1i)
                    nc.vector.tensor_add(out=q, in0=o1r, in1=o1i)
                    for ri_idx, src in ((0, p), (1, q)):
                        s2 = S3.tile([BS, H // 2, Wf], BF, tag="s3t")
                        s2f = s2.rearrange("j k l -> j (k l)")
                        for (o, n) in _nchunks(half, 512):
                            ps = PM.tile([128, 512], F32, tag="mm")
                            nc.tensor.matmul(ps[:BS, 0:n], w2_sb[:, nb, :],
                                             src[:, o:o + n], start=True, stop=True)
                            t1 = WK.tile([BS, 512], BF, tag="ssh")
                            nc.scalar.activation(out=t1[:, 0:n], in_=ps[:BS, 0:n],
                                                 func=mybir.ActivationFunctionType.Relu,
                                                 bias=b2mt_sb[:, nb:nb + 1], scale=1.0)
                            t2 = WK.tile([BS, 512], BF, tag="ssh")
                            nc.scalar.activation(out=t2[:, 0:n], in_=ps[:BS, 0:n],
                                                 func=mybir.ActivationFunctionType.Relu,
                                                 bias=b2pt_sb[:, nb:nb + 1], scale=-1.0)
                            nc.vector.tensor_sub(out=s2f[:, o:o + n],
                                                 in0=t1[:, 0:n], in1=t2[:, 0:n])
                        nc.sync.dma_start(
                            out=Ov[ri_idx, h0:h1_, :, c0:c0 + BS].rearrange("k l j -> j k l"),
                            in_=s2)

            # ================= S4: iFFT-h (selected rows) ==================
            Ofl = Od.ap().rearrange("r k l c -> r k (l c)")    # [2, H, FLATZ]
            for (o, n) in _nchunks(FLATZ, 512):
                rr = SR.tile([H, n], BF, tag="s2r")
                ri = SR.tile([H, n], BF, tag="s2r")
                nc.sync.dma_start(out=rr, in_=Ofl[0, :, o:o + n])
                nc.sync.dma_start(out=ri, in_=Ofl[1, :, o:o + n])
                ps = PM.tile([128, 512], F32, tag="mm")
                nc.tensor.matmul(ps[:2 * HH, 0:n], gc_sb, rr, start=True, stop=False)
                nc.tensor.matmul(ps[:2 * HH, 0:n], gd_sb, ri, start=False, stop=True)
                ab = ZO.tile([2 * HH, 512], BF, tag="ysb")
                nc.scalar.copy(out=ab[:, 0:n], in_=ps[:2 * HH, 0:n])
                nc.sync.dma_start(out=Ad.ap()[:, o:o + n], in_=ab[:, 0:n])

            # ======== S5+S6: iFFT-w + residual + LN2 + MLP (per group) =====
            Av = Ad.ap().rearrange("a (l c) -> a l c", l=Wf)   # [2HH, Wf, DIM]
            for g in range(8):
                xsp_keep = []
                xtg = XTG.tile([128, 6, 512], BF, tag="xtg")
                for j in range(4):
                    hl = g * 4 + j
                    ar = SR.tile([Wf, DIM], BF, tag="s5r")
                    ai = SR.tile([Wf, DIM], BF, tag="s5r")
                    nc.sync.dma_start(out=ar, in_=Av[hl, :, :].rearrange("l c -> l c"))
                    nc.sync.dma_start(out=ai, in_=Av[HH + hl, :, :].rearrange("l c -> l c"))
                    ps = PB.tile([128, DIM], F32, tag="big")
                    for (o, n) in ((0, 512), (512, 256)):
                        nc.tensor.matmul(ps[:, o:o + n], br_sb, ar[:, o:o + n],
                                         start=True, stop=False)
                        nc.tensor.matmul(ps[:, o:o + n], bi_sb, ai[:, o:o + n],
                                         start=False, stop=True)
                    xsp = XSP.tile([128, DIM], BF, tag="xsp")
                    xrt = WK.tile([128, DIM], F32, tag="xres")
                    nc.sync.dma_start(out=xrt, in_=xres_ext[hl * W:(hl + 1) * W, :])
                    nc.vector.tensor_add(out=xsp, in0=ps, in1=xrt)
                    xsp_keep.append(xsp)
                    xn2 = WK.tile([128, DIM], F32, tag="xn2")
                    nc.vector.tensor_copy(out=xn2, in_=xsp)
                    xn2b = WK.tile([128, DIM], BF, tag="xn2b")
                    layernorm(xn2, ln2w_bc, ln2b_bc, xn2b)
                    for cc in range(6):
                        pst = PM.tile([128, 128], BF, tag="mm")
                        nc.tensor.transpose(pst[:, 0:128],
                                            xn2b[:, cc * 128:(cc + 1) * 128], ident)
                        nc.scalar.copy(out=xtg[:, cc, j * 128:(j + 1) * 128],
                                       in_=pst[:, 0:128])
                # W1 + gelu
                h1_tiles = []
                for m in range(24):
                    ps = PM.tile([128, 512], F32, tag="mm")
                    for cc in range(6):
                        nc.tensor.matmul(ps, mw1_sb[cc][:, m * 128:(m + 1) * 128],
                                         xtg[:, cc, :], start=(cc == 0), stop=(cc == 5))
                    h1 = H1P.tile([128, 512], BF, tag="h1")
                    nc.scalar.activation(out=h1, in_=ps,
                                         func=mybir.ActivationFunctionType.Gelu,
                                         bias=mb1_sb[:, m:m + 1], scale=1.0)
                    h1_tiles.append(h1)
                # W2 (out channel-major) -> transpose -> +resid -> store
                for c2 in range(6):
                    ps = PM.tile([128, 512], F32, tag="mm")
                    for m in range(24):
                        nc.tensor.matmul(ps, mw2_sb[m][:, c2 * 128:(c2 + 1) * 128],
                                         h1_tiles[m], start=(m == 0), stop=(m == 23))
                    ob = WK.tile([128, 512], BF, tag="ob")
                    nc.scalar.copy(out=ob, in_=ps)
                    for j in range(4):
                        pst = PM.tile([128, 128], BF, tag="mm")
                        nc.tensor.transpose(pst[:, 0:128],
                                            ob[:, j * 128:(j + 1) * 128], ident)
                        ot = OP.tile([128, 128], F32, tag="of")
                        nc.vector.tensor_add(
                            out=ot, in0=pst[:, 0:128],
                            in1=xsp_keep[j][:, c2 * 128:(c2 + 1) * 128])
                        nc.vector.tensor_add(
                            out=ot, in0=ot,
                            in1=mb2_bc[:, c2 * 128:(c2 + 1) * 128])
                        hl = g * 4 + j
                        nc.sync.dma_start(
                            out=out_ext[hl * W:(hl + 1) * W,
                                        c2 * 128:(c2 + 1) * 128],
                            in_=ot)
    _split_excess_waits(nc)
    return nc


def _split_excess_waits(nc, cap=1):
    """HW engine instruction templates accept only 1 semaphore wait; Tile
    may attach several. Insert same-engine NoOps (each carrying one wait)
    immediately before any offending instruction."""
    for f in nc.m.functions:
        for blk in f.blocks:
            insts = blk.instructions
            out = []
            for inst in insts:
                si = getattr(inst, 'sync_info', None)
                waits = list(si.on_wait) if si is not None else []
                if len(waits) > cap:
                    extra, keep = waits[:-cap], waits[-cap:]
                    for wv in extra:
                        n = mybir.InstNoOp(
                            name=nc.get_next_instruction_name(), ins=[], outs=[])
                        n.engine = inst.engine
                        n.sync_info = mybir.SyncInfo(on_wait=[wv], on_update=[])
                        out.append(n)
                    si.on_wait = keep
                    inst.sync_info = si
                out.append(inst)
            insts[:] = out


_NC_CACHE = None


def kernel(**inputs):
    global _NC_CACHE
    x = np.asarray(inputs["x"], dtype=np.float32)          # [4, 8192, 768]
    B = x.shape[0]
    w1 = np.asarray(inputs["w1"], dtype=np.float32)
    w2 = np.asarray(inputs["w2"], dtype=np.float32)
    b1 = np.asarray(inputs["b1"], dtype=np.float32)
    b2 = np.asarray(inputs["b2"], dtype=np.float32)

    Fw = np.fft.rfft(np.eye(W), norm='ortho')
    G = np.fft.fft(np.eye(H), norm='ortho')
    Br = np.stack([np.fft.irfft(np.eye(Wf)[k], n=W, norm='ortho') for k in range(Wf)])
    Bi = np.stack([np.fft.irfft(1j * np.eye(Wf)[k], n=W, norm='ortho') for k in range(Wf)])
    Gr, Gi = np.ascontiguousarray(G.real), np.ascontiguousarray(G.imag)
    f32 = lambda a: np.ascontiguousarray(a, dtype=np.float32)

    common = {
        "fwr": f32(Fw.real), "fwi": f32(Fw.imag),
        "ga": f32(np.concatenate([Gr, Gi], 1)),
        "gb": f32(np.concatenate([-Gi, Gr], 1)),
        "brm": f32(Br), "bim": f32(Bi),
        "w1": f32(w1), "w2": f32(w2),
        "b1t": f32(b1.T),
        "b2mt": f32(b2.T - LAM), "b2pt": f32(-b2.T - LAM),
        "ln1w": f32(inputs["ln1_w"]), "ln1b": f32(inputs["ln1_b"]),
        "ln2w": f32(inputs["ln2_w"]), "ln2b": f32(inputs["ln2_b"]),
        "mb1": f32(inputs["mlp_b1"]), "mb2": f32(inputs["mlp_b2"]),
        "mw1": f32(inputs["mlp_w1"]), "mw2": f32(inputs["mlp_w2"]),
    }
    in_maps = []
    for c in range(8):
        b, halfc = c // 2, c % 2
        h0 = halfc * HH
        gsel_r = Gr[:, h0:h0 + HH]
        gsel_i = Gi[:, h0:h0 + HH]
        m = dict(common)
        m["x"] = f32(x[b])
        m["xres"] = f32(x[b, h0 * W:(h0 + HH) * W, :])
        m["gc"] = f32(np.concatenate([gsel_r, -gsel_i], 1))
        m["gd"] = f32(np.concatenate([gsel_i, gsel_r], 1))
        in_maps.append(m)

    if _NC_CACHE is None:
        _NC_CACHE = build_graph()
    res = run_bass_kernel_spmd(_NC_CACHE, in_maps, core_ids=list(range(8)))
    out = np.empty((B, H * W, DIM), dtype=np.float32)
    for c in range(8):
        b, halfc = c // 2, c % 2
        h0 = halfc * HH
        out[b, h0 * W:(h0 + HH) * W, :] = res.results[c]["out"]
    return out


if __name__ == "__main__":
    rng = np.random.default_rng(0)
    pass
